# revision 1
# baseline (speedup 1.0000x reference)
"""MultiLobeSGGX.sample() Trainium2 Bass kernel (8-core data-parallel).

Two-phase tile pipeline per core (see build_module): phase 1 computes the
luminance-based lobe routing (p_spec, mask, remapped u0s/u0d) and sin/cos of
2*pi*u1 via the ACT Sin table; phase 2 computes the SGGX VNDF specular sample
in closed form plus the cosine-free uniform-hemisphere diffuse sample, with a
mask-merged basis reconstruction shared by both lobes. Work is split across
DVE/GPSIMD per tile (DGDG pattern) with transcendentals on ACT.
"""
import sys
sys.path.insert(0, '/opt/trn_rl_repo')
import numpy as np
import concourse.bass as bass
import concourse.bacc as bacc
import concourse.mybir as mybir
import concourse.tile as tile

dt = mybir.dt.float32
A = mybir.ActivationFunctionType
OP = mybir.AluOpType
PI = float(np.pi)
EPS = 1e-6


class Scratch:
    """Manual slot reuse inside one tile-pool: fixed set of [128,F] slots."""

    def __init__(self, pool, F, n, dtype=dt):
        self.slots = [pool.tile([128, F], dtype, tag=f"sc{i}", name=f"sc{i}")
                      for i in range(n)]
        self.free = list(range(n))
        self.used = {}

    def get(self, nm):
        i = self.free.pop(0)  # FIFO: spread reuse to cut false WAR serialization
        self.used[nm] = i
        return self.slots[i][:]

    def rel(self, *nms):
        for nm in nms:
            self.free.append(self.used.pop(nm))

    def __getitem__(self, nm):
        return self.slots[self.used[nm]][:]


def build_module(R, F=512, num_devices=8, heavy_pattern=None, lum_on_g=True,
                 use_exact_rtot=True, mask_dt=mybir.dt.uint8, nsc=30,
                 p1_bufs=2, io_bufs=2, mid_mode="invert", repeat=1, CH=1):
    """R: rays per core. F: rays per partition per tile. CH: ray chunks."""
    assert R % (128 * F * CH) == 0
    T = R // (128 * F * CH)  # tiles per chunk per phase
    E = R // (128 * CH)  # columns in persistent buffers (per chunk)

    nc = bacc.Bacc("TRN2", target_bir_lowering=False, debug=False,
                   num_devices=num_devices)

    def register_const(val):
        t = nc.alloc_sbuf_tensor(f"uconst-{val}", [128, 1], dt)
        nc.gpsimd.memset(t.ap(), val)
        nc.const_aps.aps[(dt, float(val))] = t.ap()
    register_const(PI)
    register_const(PI / 2)
    register_const(float(EPS))
    register_const(-1.0)
    nc.all_engine_barrier()

    wi = nc.dram_tensor("wi", [R, 3], dt, kind="ExternalInput")
    nn = nc.dram_tensor("n", [R, 3], dt, kind="ExternalInput")
    al = nc.dram_tensor("albedo", [R, 3], dt, kind="ExternalInput")
    me = nc.dram_tensor("metallic", [R, 3], dt, kind="ExternalInput")
    ax = nc.dram_tensor("alpha_x", [R, 1], dt, kind="ExternalInput")
    sa = nc.dram_tensor("sample", [R, 2], dt, kind="ExternalInput")
    wo = nc.dram_tensor("wo", [R, 3], dt, kind="ExternalOutput")

    wi_r = wi[:, :].rearrange("(t p f) c -> t p (f c)", p=128, f=F)
    nn_r = nn[:, :].rearrange("(t p f) c -> t p (f c)", p=128, f=F)
    al_r = al[:, :].rearrange("(t p f) c -> t p (f c)", p=128, f=F)
    me_r = me[:, :].rearrange("(t p f) c -> t p (f c)", p=128, f=F)
    ax_r = ax[:, :].rearrange("(t p f) 1 -> t p f", p=128, f=F)
    sa_r = sa[:, :].rearrange("(t p f) c -> t p (f c)", p=128, f=F)
    wo_r = wo[:, :].rearrange("(t p f) c -> t p (f c)", p=128, f=F)

    if heavy_pattern is None:
        heavy_pattern = ["D"] * T

    ve = nc.vector
    ge = nc.gpsimd
    ae = nc.scalar

    with tile.TileContext(nc) as tc:
        with tc.tile_pool(name="persist", bufs=1) as pp:
            sinb = pp.tile([128, E], dt, tag="sinb", name="sinb")
            cosb = pp.tile([128, E], dt, tag="cosb", name="cosb")
            u0sb = pp.tile([128, E], dt, tag="u0sb", name="u0sb")
            u0db = pp.tile([128, E], dt, tag="u0db", name="u0db")
            maskb = pp.tile([128, E], mask_dt, tag="maskb", name="maskb")

            # ---------------- Phase 1 ----------------
            for rep_ch in range(repeat * CH):
              ch = rep_ch % CH
              with tc.tile_pool(name="p1", bufs=p1_bufs) as p1:
                for t0_ in range(T):
                    t = t0_ % T
                    tg = ch * T + t
                    heavy = ve if heavy_pattern[t] == "D" else ge
                    light = ge if heavy_pattern[t] == "D" else ve
                    lum_e = light if lum_on_g else heavy
                    sl = slice(t * F, (t + 1) * F)

                    AL = p1.tile([128, 3 * F], dt, tag="AL", name="AL")
                    ME = p1.tile([128, 3 * F], dt, tag="ME", name="ME")
                    S = p1.tile([128, 2 * F], dt, tag="S", name="S")
                    nc.sync.dma_start(AL[:], al_r[tg])
                    nc.sync.dma_start(ME[:], me_r[tg])
                    nc.sync.dma_start(S[:], sa_r[tg])
                    ALv = AL[:].rearrange("p (f c) -> p f c", c=3)
                    MEv = ME[:].rearrange("p (f c) -> p f c", c=3)
                    Sv = S[:].rearrange("p (f c) -> p f c", c=2)
                    s0 = Sv[:, :, 0]
                    s1 = Sv[:, :, 1]

                    def nt(tag):
                        return p1.tile([128, F], dt, tag=tag, name=tag)

                    w_d = nt("w_d"); w_s = nt("w_s"); tmp = nt("p1tmp")
                    def lum(out, V):
                        if lum_e is ve:
                            lum_e.tensor_scalar(tmp[:], V[:, :, 1], 0.7152, None, op0=OP.mult)
                            lum_e.scalar_tensor_tensor(out[:], V[:, :, 0], 0.2126, tmp[:], OP.mult, OP.add)
                            lum_e.scalar_tensor_tensor(out[:], V[:, :, 2], 0.0722, out[:], OP.mult, OP.add)
                        else:
                            tmp2 = nt("p1tmp2")
                            lum_e.tensor_scalar(tmp[:], V[:, :, 0], 0.2126, None, op0=OP.mult)
                            lum_e.tensor_scalar(tmp2[:], V[:, :, 1], 0.7152, None, op0=OP.mult)
                            lum_e.tensor_tensor(tmp[:], tmp[:], tmp2[:], OP.add)
                            lum_e.tensor_scalar(tmp2[:], V[:, :, 2], 0.0722, None, op0=OP.mult)
                            lum_e.tensor_tensor(out[:], tmp2[:], tmp[:], OP.add)
                    lum(w_d, ALv)
                    lum(w_s, MEv)

                    tote = nt("tote")
                    if mid_mode == "invert":
                        mid1 = ge if heavy_pattern[t] == "D" else ve
                    else:
                        mid1 = heavy
                    mid1.tensor_tensor(tote[:], w_d[:], w_s[:], OP.add)
                    ae.activation(tote[:], tote[:], A.Identity, bias=float(EPS))
                    rt = nt("rt")
                    if use_exact_rtot:
                        ve.reciprocal(rt[:], tote[:])
                    else:
                        ve.reciprocal_approx_accurate(rt[:], tote[:], tmp[:])
                    p = nt("p")
                    heavy.tensor_tensor(p[:], w_s[:], rt[:], OP.mult)
                    ve.tensor_tensor(maskb[:, sl], p[:], s0, OP.is_gt)

                    pm = nt("pm")
                    light.tensor_scalar(pm[:], p[:], EPS, None, op0=OP.max)
                    rpm = nt("rpm")
                    ve.reciprocal_approx_accurate(rpm[:], pm[:], tmp[:])
                    u0s = nt("u0s")
                    mid1.tensor_tensor(u0s[:], s0, rpm[:], OP.mult)
                    light.tensor_scalar(u0sb[:, sl], u0s[:], 0.0, 1.0, op0=OP.max, op1=OP.min)

                    dsp = nt("dsp")
                    heavy.tensor_tensor(dsp[:], s0, p[:], OP.subtract)
                    om = nt("om")
                    ae.activation(om[:], p[:], A.Identity, scale=-1.0, bias=1.0)
                    light.tensor_scalar(om[:], om[:], EPS, None, op0=OP.max)
                    rom = nt("rom")
                    ve.reciprocal_approx_accurate(rom[:], om[:], tmp[:])
                    u0d = nt("u0d")
                    mid1.tensor_tensor(u0d[:], dsp[:], rom[:], OP.mult)
                    light.tensor_scalar(u0db[:, sl], u0d[:], 0.0, 1.0, op0=OP.max, op1=OP.min)

                    # sin(2*pi*s1) = Sin(-2*pi*s1 + pi)
                    ae.activation(sinb[:, sl], s1, A.Sin, scale=-2.0 * PI, bias=PI)
                    # cos(2*pi*s1) = Sin(-2*pi*(s1 - [s1>0.75]) + pi/2)
                    wadj = nt("wadj")
                    light.tensor_scalar(wadj[:], s1, 0.75, None, op0=OP.is_gt)
                    carg = nt("carg")
                    light.tensor_tensor(carg[:], s1, wadj[:], OP.subtract)
                    ae.activation(cosb[:, sl], carg[:], A.Sin, scale=-2.0 * PI, bias=PI / 2)

            # ---------------- Phase 2 ----------------
              with (
                tc.tile_pool(name="p2io", bufs=io_bufs) as pio,
                tc.tile_pool(name="p2sc", bufs=1) as psc,
              ):
                SC = Scratch(psc, F, nsc)
                for t0_ in range(T):
                    t = t0_ % T
                    tg = ch * T + t
                    heavy = ve if heavy_pattern[t] == "D" else ge
                    light = ge if heavy_pattern[t] == "D" else ve
                    if mid_mode == "invert":
                        mid = ge if heavy_pattern[t] == "D" else ve
                    else:
                        mid = heavy
                    sl = slice(t * F, (t + 1) * F)

                    WI = pio.tile([128, 3 * F], dt, tag="WI", name="WI")
                    NN = pio.tile([128, 3 * F], dt, tag="NN", name="NN")
                    ALP = pio.tile([128, F], dt, tag="ALP", name="ALP")
                    OUT = pio.tile([128, 3 * F], dt, tag="OUT", name="OUT")
                    nc.scalar.dma_start(WI[:], wi_r[tg])
                    nc.scalar.dma_start(NN[:], nn_r[tg])
                    nc.sync.dma_start(ALP[:], ax_r[tg])
                    WIv = WI[:].rearrange("p (f c) -> p f c", c=3)
                    NNv = NN[:].rearrange("p (f c) -> p f c", c=3)
                    OUTv = OUT[:].rearrange("p (f c) -> p f c", c=3)
                    nx = NNv[:, :, 0]; ny = NNv[:, :, 1]; nz = NNv[:, :, 2]
                    mask = maskb[:, sl]

                    # b = mask ? wi : n
                    bx = SC.get("bx"); by = SC.get("by"); bz = SC.get("bz")
                    for c, bt in ((0, bx), (1, by), (2, bz)):
                        ae.activation(bt, NNv[:, :, c], A.Copy)
                        ve.copy_predicated(bt, mask, WIv[:, :, c])

                    sgn = SC.get("sgn")
                    ae.activation(sgn, bz, A.Sign)
                    den = SC.get("den")
                    mid.tensor_tensor(den, sgn, bz, OP.add)
                    rb = SC.get("rb")
                    ve.reciprocal_approx_accurate(rb, den, SC.get("rsc"))
                    SC.rel("den", "rsc")
                    u = SC.get("u")
                    mid.tensor_tensor(u, sgn, bx, OP.mult)

                    # dots (k_n, j_n, i_n)
                    m1 = SC.get("m1"); m2_ = SC.get("m2"); d2 = SC.get("d2")
                    heavy.tensor_tensor(m1, bx, nx, OP.mult)
                    heavy.tensor_tensor(m2_, by, ny, OP.mult)
                    heavy.tensor_tensor(d2, m1, m2_, OP.add)
                    SC.rel("m1", "m2")
                    Q = SC.get("Q")
                    heavy.tensor_tensor(Q, d2, rb, OP.mult)
                    heavy.tensor_tensor(Q, Q, nz, OP.add)
                    kn = SC.get("kn")
                    heavy.tensor_tensor(kn, u, Q, OP.mult)
                    heavy.tensor_tensor(kn, nx, kn, OP.subtract)
                    jn = SC.get("jn"); jq = SC.get("jq")
                    heavy.tensor_tensor(jn, sgn, ny, OP.mult)
                    heavy.tensor_tensor(jq, by, Q, OP.mult)
                    heavy.tensor_tensor(jn, jn, jq, OP.subtract)
                    SC.rel("jq", "Q")
                    inn = SC.get("inn")
                    heavy.tensor_tensor(inn, bz, nz, OP.mult)
                    heavy.tensor_tensor(inn, d2, inn, OP.add)
                    SC.rel("d2")

                    a2 = SC.get("a2"); beta = SC.get("beta")
                    ae.activation(a2, ALP[:], A.Square)
                    ae.activation(beta, a2, A.Identity, scale=-1.0, bias=1.0)

                    bk = SC.get("bk"); bkk = SC.get("bkk")
                    mid.tensor_tensor(bk, beta, kn, OP.mult)
                    mid.tensor_tensor(bkk, bk, kn, OP.mult)
                    SC.rel("kn")
                    m2t = SC.get("m2t")
                    ae.activation(m2t, bkk, A.Identity, scale=-1.0, bias=1.0)
                    SC.rel("bkk")
                    mS = SC.get("mS")
                    ae.activation(mS, m2t, A.Sqrt)
                    SC.rel("m2t")
                    bi = SC.get("bi"); bii = SC.get("bii")
                    mid.tensor_tensor(bi, beta, inn, OP.mult)
                    SC.rel("beta")
                    mid.tensor_tensor(bii, bi, inn, OP.mult)
                    Sii = SC.get("Sii")
                    mid.tensor_tensor(Sii, bii, a2, OP.add)
                    SC.rel("bii", "a2")
                    sqii = SC.get("sqii")
                    ae.activation(sqii, Sii, A.Sqrt)
                    q = SC.get("q"); rm = SC.get("rm")
                    ve.reciprocal_approx_accurate(q, sqii, SC.get("rsc"))
                    SC.rel("sqii", "rsc")
                    ve.reciprocal_approx_accurate(rm, mS, SC.get("rsc2"))
                    SC.rel("rsc2")
                    bkj = SC.get("bkj"); bki = SC.get("bki"); bij = SC.get("bij")
                    mid.tensor_tensor(bkj, bk, jn, OP.mult)
                    mid.tensor_tensor(bki, bk, inn, OP.mult)
                    SC.rel("bk")
                    mid.tensor_tensor(bij, bi, jn, OP.mult)
                    SC.rel("bi", "jn", "inn")

                    squ = SC.get("squ")
                    ae.activation(squ, u0sb[:, sl], A.Sqrt)
                    squa = SC.get("squa")
                    mid.tensor_tensor(squa, squ, ALP[:], OP.mult)
                    SC.rel("squ")
                    uu = SC.get("uu"); vv = SC.get("vv")
                    mid.tensor_tensor(uu, squa, cosb[:, sl], OP.mult)
                    mid.tensor_tensor(vv, squa, sinb[:, sl], OP.mult)
                    SC.rel("squa")
                    omu = SC.get("omu")
                    ae.activation(omu, u0sb[:, sl], A.Identity, scale=-1.0, bias=1.0)
                    ww = SC.get("ww")
                    ae.activation(ww, omu, A.Sqrt)
                    SC.rel("omu")

                    vvq = SC.get("vvq"); wwq = SC.get("wwq")
                    mid.tensor_tensor(vvq, vv, q, OP.mult)
                    SC.rel("vv")
                    mid.tensor_tensor(wwq, ww, q, OP.mult)
                    SC.rel("ww", "q")
                    tb = SC.get("tb")
                    c0 = SC.get("c0"); c1 = SC.get("c1"); c2 = SC.get("c2")
                    heavy.tensor_tensor(tb, vvq, bkj, OP.mult)
                    SC.rel("bkj")
                    heavy.tensor_tensor(tb, uu, tb, OP.add)
                    SC.rel("uu")
                    heavy.tensor_tensor(tb, tb, rm, OP.mult)
                    SC.rel("rm")
                    heavy.tensor_tensor(c0, wwq, bki, OP.mult)
                    SC.rel("bki")
                    heavy.tensor_tensor(c0, tb, c0, OP.add)
                    heavy.tensor_tensor(c1, vvq, mS, OP.mult)
                    SC.rel("vvq", "mS")
                    heavy.tensor_tensor(tb, wwq, bij, OP.mult)
                    SC.rel("bij")
                    heavy.tensor_tensor(c1, c1, tb, OP.add)
                    SC.rel("tb")
                    heavy.tensor_tensor(c2, wwq, Sii, OP.mult)
                    SC.rel("wwq", "Sii")

                    c0s = SC.get("c0s"); c1s = SC.get("c1s"); c2s = SC.get("c2s")
                    ae.activation(c0s, c0, A.Square)
                    ae.activation(c1s, c1, A.Square)
                    ae.activation(c2s, c2, A.Square)
                    n2 = SC.get("n2")
                    heavy.tensor_tensor(n2, c0s, c1s, OP.add)
                    heavy.tensor_tensor(n2, n2, c2s, OP.add)
                    SC.rel("c0s", "c1s", "c2s")
                    r2 = SC.get("r2")
                    ve.reciprocal_approx_fast(r2, n2)
                    SC.rel("n2")
                    tq = SC.get("tq")
                    heavy.tensor_tensor(tq, c2, r2, OP.mult)
                    SC.rel("r2")
                    d0 = SC.get("d0"); d1 = SC.get("d1"); d2p = SC.get("d2p")
                    ve.scalar_tensor_tensor(d0, tq, 2.0, c0, OP.mult, OP.mult)
                    SC.rel("c0")
                    ve.scalar_tensor_tensor(d1, tq, 2.0, c1, OP.mult, OP.mult)
                    SC.rel("c1")
                    ve.scalar_tensor_tensor(d2p, tq, 2.0, c2, OP.mult, OP.mult)
                    SC.rel("c2", "tq")
                    ae.activation(d2p, d2p, A.Identity, bias=-1.0)

                    # diffuse coeffs -> e0/e1/e2 bases, then select spec via CP
                    z2 = SC.get("z2")
                    ae.activation(z2, u0db[:, sl], A.Square)
                    omz = SC.get("omz")
                    ae.activation(omz, z2, A.Identity, scale=-1.0, bias=1.0)
                    SC.rel("z2")
                    rd = SC.get("rd")
                    ae.activation(rd, omz, A.Sqrt)
                    SC.rel("omz")
                    e0 = SC.get("e0"); e1 = SC.get("e1")
                    mid.tensor_tensor(e0, rd, cosb[:, sl], OP.mult)
                    mid.tensor_tensor(e1, rd, sinb[:, sl], OP.mult)
                    SC.rel("rd")
                    ve.copy_predicated(e0, mask, d0)
                    SC.rel("d0")
                    ve.copy_predicated(e1, mask, d1)
                    SC.rel("d1")
                    e2 = u0db[:, sl]
                    ve.copy_predicated(e2, mask, d2p)
                    SC.rel("d2p")

                    # recon: wo = e0*t1(b) + e1*t2(b) + e2*b
                    H = SC.get("H"); Ht = SC.get("Ht"); G = SC.get("G")
                    heavy.tensor_tensor(H, u, e0, OP.mult)
                    heavy.tensor_tensor(Ht, by, e1, OP.mult)
                    heavy.tensor_tensor(H, H, Ht, OP.add)
                    heavy.tensor_tensor(G, H, rb, OP.mult)
                    SC.rel("rb")
                    heavy.tensor_tensor(G, G, e2, OP.subtract)
                    gx = SC.get("gx")
                    heavy.tensor_tensor(gx, bx, G, OP.mult)
                    heavy.tensor_tensor(OUTv[:, :, 0], e0, gx, OP.subtract)
                    SC.rel("e0")
                    heavy.tensor_tensor(gx, sgn, e1, OP.mult)
                    SC.rel("sgn")
                    heavy.tensor_tensor(Ht, by, G, OP.mult)
                    heavy.tensor_tensor(OUTv[:, :, 1], gx, Ht, OP.subtract)
                    heavy.tensor_tensor(gx, e2, bz, OP.mult)
                    heavy.tensor_tensor(OUTv[:, :, 2], gx, H, OP.subtract)
                    SC.rel("bx", "by", "bz", "u", "e1", "H", "Ht", "G", "gx")

                    nc.scalar.dma_start(wo_r[tg], OUT[:])

    nc.compile()
    return nc




# ---------------- host runner (self-contained deliverable) ----------------
NCORES = 8
_CACHE = {}


def _get_module(R):
    if R not in _CACHE:
        T = max(1, R // (128 * 512))
        _CACHE[R] = build_module(
            R, F=512, num_devices=NCORES, nsc=58, io_bufs=1, lum_on_g=False,
            heavy_pattern=(list("DGDGDGDG") * ((T + 7) // 8))[:T],
        )
    return _CACHE[R]


def kernel(wi, n, albedo, metallic, alpha_x, alpha_y, sample):
    """Full-input MultiLobeSGGX sample(): shards rays across 8 NeuronCores,
    runs the Bass kernel, gathers the full [N,3] float32 output.
    alpha_y is unused (the module asserts alpha_x == alpha_y)."""
    from concourse.bass_utils import run_bass_kernel_spmd
    wi = np.ascontiguousarray(wi, dtype=np.float32)
    n = np.ascontiguousarray(n, dtype=np.float32)
    albedo = np.ascontiguousarray(albedo, dtype=np.float32)
    metallic = np.ascontiguousarray(metallic, dtype=np.float32)
    alpha_x = np.ascontiguousarray(alpha_x, dtype=np.float32)
    sample = np.ascontiguousarray(sample, dtype=np.float32)
    Nf = wi.shape[0]
    R = Nf // NCORES
    nc = _get_module(R)
    in_maps = []
    for c in range(NCORES):
        s = slice(c * R, (c + 1) * R)
        in_maps.append({
            "wi": wi[s], "n": n[s], "albedo": albedo[s],
            "metallic": metallic[s], "alpha_x": alpha_x[s], "sample": sample[s],
        })
    res = run_bass_kernel_spmd(nc, in_maps, core_ids=list(range(NCORES)))
    return np.concatenate([res.results[c]["wo"] for c in range(NCORES)], axis=0)



# revision 2
# speedup vs baseline: 1.4198x; 1.4198x over previous
"""MultiLobeSGGX.sample() Trainium2 Bass kernel v3.

Single software-pipelined loop: iteration t emits routing (phase A) for tile
t and sampling (phase B) for tile t-1, so the scheduler can overlap the
Pool-heavy routing with the DVE-heavy sampling across tiles. Routing results
pass through a small ring of per-tile tiles instead of whole-core persistent
buffers. fp32 routing for an exact mask; fp16 sampling on DVE 2x mode.
"""
import sys
sys.path.insert(0, '/opt/trn_rl_repo')
import numpy as np
import concourse.bass as bass
import concourse.bacc as bacc
import concourse.mybir as mybir
import concourse.tile as tile

f32 = mybir.dt.float32
f16 = mybir.dt.float16
u8 = mybir.dt.uint8
A = mybir.ActivationFunctionType
OP = mybir.AluOpType
PI = float(np.pi)

ASSIGN = dict(
    lum_d='P', lum_s='P', tote='P', st='P', mask='D',
    u0s='P', u0num='P', u0d='P', wadj='P', carg='P',
    ncopy='A', bcopy='A', alp16='X', sgn='A', squa='P',
    dots='D', alpha='D', a2='A', uvw='D', cblock='D', n2='D', dchain='D',
    diff_z2='A', diff_e0='D', diff_e1='D',
    recon='D', out0='P', out1='P', out2='P',
)


class Scratch:
    def __init__(self, pool, F, n, dtype=f16, pfx="sc"):
        self.slots = [pool.tile([128, F], dtype, tag=f"{pfx}{i}", name=f"{pfx}{i}")
                      for i in range(n)]
        self.free = list(range(n))
        self.used = {}

    def get(self, nm):
        i = self.free.pop(0)
        self.used[nm] = i
        return self.slots[i][:]

    def rel(self, *nms):
        for nm in nms:
            self.free.append(self.used.pop(nm))

    def __getitem__(self, nm):
        return self.slots[self.used[nm]][:]


def build_module(R, F=512, num_devices=8, assign=None, nsc=26, nscsets=3,
                 io_bufs=3, a_bufs=3, ring=4):
    asg = dict(ASSIGN)
    if assign:
        asg.update(assign)
    assert R % (128 * F) == 0
    T = R // (128 * F)

    nc = bacc.Bacc("TRN2", target_bir_lowering=False, debug=False,
                   num_devices=num_devices)

    def register_const(val):
        t = nc.alloc_sbuf_tensor(f"uconst-{val}", [128, 1], f32)
        nc.gpsimd.memset(t.ap(), val)
        nc.const_aps.aps[(f32, float(val))] = t.ap()
    register_const(PI)
    register_const(PI / 2)
    nc.all_engine_barrier()

    wi = nc.dram_tensor("wi", [R, 3], f32, kind="ExternalInput")
    nn = nc.dram_tensor("n", [R, 3], f32, kind="ExternalInput")
    al = nc.dram_tensor("albedo", [R, 3], f32, kind="ExternalInput")
    me = nc.dram_tensor("metallic", [R, 3], f32, kind="ExternalInput")
    ax = nc.dram_tensor("alpha_x", [R, 1], f32, kind="ExternalInput")
    sa = nc.dram_tensor("sample", [R, 2], f32, kind="ExternalInput")
    wo = nc.dram_tensor("wo", [R, 3], f32, kind="ExternalOutput")

    wi_r = wi[:, :].rearrange("(t p f) c -> t p (f c)", p=128, f=F)
    nn_r = nn[:, :].rearrange("(t p f) c -> t p (f c)", p=128, f=F)
    al_r = al[:, :].rearrange("(t p f) c -> t p (f c)", p=128, f=F)
    me_r = me[:, :].rearrange("(t p f) c -> t p (f c)", p=128, f=F)
    ax_r = ax[:, :].rearrange("(t p f) 1 -> t p f", p=128, f=F)
    sa_r = sa[:, :].rearrange("(t p f) c -> t p (f c)", p=128, f=F)
    wo_r = wo[:, :].rearrange("(t p f) c -> t p (f c)", p=128, f=F)

    ve, ge, ae = nc.vector, nc.gpsimd, nc.scalar
    ENG = {'D': ve, 'P': ge, 'A': ae}

    def eng(g):
        return ENG[asg[g]]

    with tile.TileContext(nc) as tc:
        with (
            tc.tile_pool(name="ring", bufs=ring) as pr,
            tc.tile_pool(name="pA", bufs=a_bufs) as p1,
            tc.tile_pool(name="pBio", bufs=io_bufs) as pio,
            tc.tile_pool(name="pBsc", bufs=1) as psc,
        ):
            SCs = [Scratch(psc, F, nsc, f16, pfx=f"s{i}") for i in range(nscsets)]

            def phase_a(tg):
                """Routing for tile tg -> ring tiles (sin,cos,u0s,u0d fp16, mask u8)."""
                AL = p1.tile([128, 3 * F], f32, tag="AL", name="AL")
                ME = p1.tile([128, 3 * F], f32, tag="ME", name="ME")
                S = p1.tile([128, 2 * F], f32, tag="S", name="S")
                nc.sync.dma_start(AL[:], al_r[tg])
                nc.sync.dma_start(ME[:], me_r[tg])
                nc.sync.dma_start(S[:], sa_r[tg])
                ALv = AL[:].rearrange("p (f c) -> p f c", c=3)
                MEv = ME[:].rearrange("p (f c) -> p f c", c=3)
                Sv = S[:].rearrange("p (f c) -> p f c", c=2)
                s0 = Sv[:, :, 0]
                s1 = Sv[:, :, 1]

                sin_t = pr.tile([128, F], f16, tag="sin", name="sin")
                cos_t = pr.tile([128, F], f16, tag="cos", name="cos")
                u0s_t = pr.tile([128, F], f16, tag="u0s", name="u0s")
                u0d_t = pr.tile([128, F], f16, tag="u0d", name="u0d")
                msk_t = pr.tile([128, F], u8, tag="msk", name="msk")

                def nt(tag):
                    return p1.tile([128, F], f32, tag=tag, name=tag)

                w_d = nt("w_d"); w_s = nt("w_s")
                td = nt("td"); ts = nt("ts")

                def lum(e, out, tmp, V):
                    if e is ve:
                        e.tensor_scalar(tmp[:], V[:, :, 1], 0.7152, None, op0=OP.mult)
                        e.scalar_tensor_tensor(out[:], V[:, :, 0], 0.2126, tmp[:], OP.mult, OP.add)
                        e.scalar_tensor_tensor(out[:], V[:, :, 2], 0.0722, out[:], OP.mult, OP.add)
                    else:
                        tmp2 = nt("lt2")
                        e.tensor_scalar(tmp[:], V[:, :, 0], 0.2126, None, op0=OP.mult)
                        e.tensor_scalar(tmp2[:], V[:, :, 1], 0.7152, None, op0=OP.mult)
                        e.tensor_tensor(tmp[:], tmp[:], tmp2[:], OP.add)
                        e.tensor_scalar(tmp2[:], V[:, :, 2], 0.0722, None, op0=OP.mult)
                        e.tensor_tensor(out[:], tmp2[:], tmp[:], OP.add)
                lum(eng('lum_d'), w_d, td, ALv)
                lum(eng('lum_s'), w_s, ts, MEv)

                tote = nt("tote"); st = nt("st")
                if asg['tote'] == 'D':
                    ve.scalar_tensor_tensor(tote[:], w_d[:], 1e-6, w_s[:], OP.add, OP.add)
                else:
                    eng('tote').tensor_scalar(tote[:], w_d[:], 1e-6, None, op0=OP.add)
                    eng('tote').tensor_tensor(tote[:], tote[:], w_s[:], OP.add)
                eng('st').tensor_tensor(st[:], s0, tote[:], OP.mult)
                eng('mask').tensor_tensor(msk_t[:], w_s[:], st[:], OP.is_gt)

                rw = nt("rw")
                ve.reciprocal(rw[:], w_s[:])
                eng('u0s').tensor_tensor(u0s_t[:], st[:], rw[:], OP.mult)
                num = nt("num"); rd_ = nt("rd_")
                eng('u0num').tensor_tensor(num[:], st[:], w_s[:], OP.subtract)
                ve.reciprocal(rd_[:], w_d[:])
                eng('u0d').tensor_tensor(u0d_t[:], num[:], rd_[:], OP.mult)
                ve.tensor_scalar(u0d_t[:], u0d_t[:], 0.0, 1.0, op0=OP.max, op1=OP.min)

                ae.activation(sin_t[:], s1, A.Sin, scale=-2.0 * PI, bias=PI)
                wadj = nt("wadj"); carg = nt("carg")
                eng('wadj').tensor_scalar(wadj[:], s1, 0.75, None, op0=OP.is_gt)
                eng('carg').tensor_tensor(carg[:], s1, wadj[:], OP.subtract)
                ae.activation(cos_t[:], carg[:], A.Sin, scale=-2.0 * PI, bias=PI / 2)
                return dict(sin=sin_t, cos=cos_t, u0s=u0s_t, u0d=u0d_t, msk=msk_t)

            def phase_b(tg, rt, SC):
                """Sampling for tile tg from ring tiles rt."""
                mask = rt['msk'][:]
                sinb = rt['sin'][:]
                cosb = rt['cos'][:]
                u0sb = rt['u0s'][:]
                u0db = rt['u0d'][:]

                WI = pio.tile([128, 3 * F], f32, tag="WI", name="WI")
                NN = pio.tile([128, 3 * F], f32, tag="NN", name="NN")
                ALP = pio.tile([128, F], f32, tag="ALP", name="ALP")
                OUT = pio.tile([128, 3 * F], f32, tag="OUT", name="OUT")
                nc.sync.dma_start(WI[:], wi_r[tg])
                nc.sync.dma_start(NN[:], nn_r[tg])
                nc.sync.dma_start(ALP[:], ax_r[tg])
                WIv = WI[:].rearrange("p (f c) -> p f c", c=3)
                NNv = NN[:].rearrange("p (f c) -> p f c", c=3)
                OUTv = OUT[:].rearrange("p (f c) -> p f c", c=3)

                nx = SC.get("nx"); ny = SC.get("ny"); nz = SC.get("nz")
                enc = eng('ncopy')
                for c, t_ in ((0, nx), (1, ny), (2, nz)):
                    if asg['ncopy'] == 'A':
                        ae.activation(t_, NNv[:, :, c], A.Copy)
                    else:
                        enc.tensor_copy(t_, NNv[:, :, c])
                bx = SC.get("bx"); by = SC.get("by"); bz = SC.get("bz")
                ebc = eng('bcopy')
                for src, t_ in ((nx, bx), (ny, by), (nz, bz)):
                    if asg['bcopy'] == 'A':
                        ae.activation(t_, src, A.Copy)
                    else:
                        ebc.tensor_copy(t_, src)
                for c, t_ in ((0, bx), (1, by), (2, bz)):
                    ve.copy_predicated(t_, mask, WIv[:, :, c])

                sgn = SC.get("sgn")
                if asg['sgn'] == 'A':
                    ae.activation(sgn, bz, A.Sign)
                else:
                    ve.tensor_scalar(sgn, bz, 0.0, None, op0=OP.is_ge)
                    ve.tensor_scalar(sgn, sgn, 2.0, -1.0, op0=OP.mult, op1=OP.add)
                den = SC.get("den"); rb = SC.get("rb")
                ve.tensor_tensor(den, sgn, bz, OP.add)
                with nc.allow_low_precision(reason="den in +-[1,2]"):
                    ve.reciprocal(rb, den)
                SC.rel("den")
                u = SC.get("u")
                ve.tensor_tensor(u, sgn, bx, OP.mult)

                ed_ = eng('dots')
                m1 = SC.get("m1"); m2_ = SC.get("m2"); d2 = SC.get("d2")
                ed_.tensor_tensor(m1, bx, nx, OP.mult)
                ed_.tensor_tensor(m2_, by, ny, OP.mult)
                ed_.tensor_tensor(d2, m1, m2_, OP.add)
                SC.rel("m1", "m2")
                inn = SC.get("inn")
                ed_.tensor_tensor(inn, bz, nz, OP.mult)
                ed_.tensor_tensor(inn, d2, inn, OP.add)
                Q = SC.get("Q")
                ed_.tensor_tensor(Q, d2, rb, OP.mult)
                ed_.tensor_tensor(Q, Q, nz, OP.add)
                SC.rel("d2")
                kn = SC.get("kn")
                ed_.tensor_tensor(kn, u, Q, OP.mult)
                ed_.tensor_tensor(kn, nx, kn, OP.subtract)
                jn = SC.get("jn"); jq = SC.get("jq")
                ed_.tensor_tensor(jn, sgn, ny, OP.mult)
                ed_.tensor_tensor(jq, by, Q, OP.mult)
                ed_.tensor_tensor(jn, jn, jq, OP.subtract)
                SC.rel("jq", "Q", "nx", "ny", "nz")

                ea = eng('alpha')
                a2 = SC.get("a2")
                if asg['a2'] == 'A':
                    ae.activation(a2, ALP[:], A.Square)
                else:
                    eng('a2').tensor_tensor(a2, ALP[:], ALP[:], OP.mult)
                beta = SC.get("beta")
                ve.tensor_scalar(beta, a2, -1.0, 1.0, op0=OP.mult, op1=OP.add)
                bk = SC.get("bk"); bkk = SC.get("bkk")
                ea.tensor_tensor(bk, beta, kn, OP.mult)
                ea.tensor_tensor(bkk, bk, kn, OP.mult)
                SC.rel("kn")
                m2t = SC.get("m2t")
                ve.tensor_scalar(m2t, bkk, -1.0, 1.0, op0=OP.mult, op1=OP.add)
                ve.tensor_scalar(m2t, m2t, 1e-4, None, op0=OP.max)
                SC.rel("bkk")
                bi = SC.get("bi"); bii = SC.get("bii")
                ea.tensor_tensor(bi, beta, inn, OP.mult)
                SC.rel("beta")
                ea.tensor_tensor(bii, bi, inn, OP.mult)
                Sii = SC.get("Sii")
                ea.tensor_tensor(Sii, bii, a2, OP.add)
                SC.rel("bii", "a2")
                bkj = SC.get("bkj"); bki = SC.get("bki"); bij = SC.get("bij")
                ea.tensor_tensor(bkj, bk, jn, OP.mult)
                ea.tensor_tensor(bki, bk, inn, OP.mult)
                SC.rel("bk")
                ea.tensor_tensor(bij, bi, jn, OP.mult)
                SC.rel("bi", "jn", "inn")

                q = SC.get("q"); rm = SC.get("rm"); mS = SC.get("mS")
                sqii = SC.get("sqii")
                ae.activation(sqii, Sii, A.Sqrt)
                ae.activation(mS, m2t, A.Sqrt)
                SC.rel("m2t")
                with nc.allow_low_precision(reason="vndf normalization"):
                    ve.reciprocal(q, sqii)
                    ve.reciprocal(rm, mS)
                SC.rel("sqii")
                euv = eng('uvw')

                squ = SC.get("squ"); squa = SC.get("squa")
                ae.activation(squ, u0sb, A.Sqrt)
                eng('squa').tensor_tensor(squa, squ, ALP[:], OP.mult)
                SC.rel("squ")
                uu = SC.get("uu"); vv = SC.get("vv"); ww = SC.get("ww")
                euv.tensor_tensor(uu, squa, cosb, OP.mult)
                euv.tensor_tensor(vv, squa, sinb, OP.mult)
                SC.rel("squa")
                ae.activation(ww, u0sb, A.Sqrt, scale=-1.0, bias=1.0)

                ec = eng('cblock')
                vvq = SC.get("vvq"); wwq = SC.get("wwq")
                ec.tensor_tensor(vvq, vv, q, OP.mult)
                ec.tensor_tensor(wwq, ww, q, OP.mult)
                SC.rel("ww", "q")
                tb = SC.get("tb")
                c0 = SC.get("c0"); c1 = SC.get("c1"); c2 = SC.get("c2")
                ec.tensor_tensor(tb, vvq, bkj, OP.mult)
                SC.rel("bkj")
                ec.tensor_tensor(tb, uu, tb, OP.add)
                SC.rel("uu")
                ec.tensor_tensor(tb, tb, rm, OP.mult)
                SC.rel("rm")
                ec.tensor_tensor(c0, wwq, bki, OP.mult)
                SC.rel("bki")
                ec.tensor_tensor(c0, tb, c0, OP.add)
                ec.tensor_tensor(c1, vvq, mS, OP.mult)
                SC.rel("vvq", "mS")
                ec.tensor_tensor(tb, wwq, bij, OP.mult)
                SC.rel("bij")
                ec.tensor_tensor(c1, c1, tb, OP.add)
                SC.rel("tb")
                ec.tensor_tensor(c2, wwq, Sii, OP.mult)
                SC.rel("wwq", "Sii")

                en = eng('n2')
                c0s = SC.get("c0s"); c1s = SC.get("c1s"); c2s = SC.get("c2s")
                en.tensor_tensor(c0s, c0, c0, OP.mult)
                en.tensor_tensor(c1s, c1, c1, OP.mult)
                en.tensor_tensor(c2s, c2, c2, OP.mult)
                n2 = SC.get("n2")
                en.tensor_tensor(n2, c0s, c1s, OP.add)
                en.tensor_tensor(n2, n2, c2s, OP.add)
                SC.rel("c0s", "c1s", "c2s")
                r22 = SC.get("r22")
                with nc.allow_low_precision(reason="|c|^2 >= alpha^2"):
                    ve.reciprocal(r22, n2)
                SC.rel("n2")
                edc = eng('dchain')
                tq2 = SC.get("tq2")
                ve.tensor_scalar(tq2, c2, 2.0, None, op0=OP.mult)
                edc.tensor_tensor(tq2, tq2, r22, OP.mult)
                SC.rel("r22")
                d0 = SC.get("d0"); d1 = SC.get("d1"); d2p = SC.get("d2p")
                edc.tensor_tensor(d0, tq2, c0, OP.mult)
                SC.rel("c0")
                edc.tensor_tensor(d1, tq2, c1, OP.mult)
                SC.rel("c1")
                edc.tensor_tensor(d2p, tq2, c2, OP.mult)
                SC.rel("c2", "tq2")
                ve.tensor_scalar(d2p, d2p, -1.0, None, op0=OP.add)

                z2 = SC.get("z2"); rdt = SC.get("rdt")
                if asg['diff_z2'] == 'A':
                    ae.activation(z2, u0db, A.Square)
                else:
                    eng('diff_z2').tensor_tensor(z2, u0db, u0db, OP.mult)
                ve.tensor_scalar(z2, z2, 1.0, None, op0=OP.min)
                ae.activation(rdt, z2, A.Sqrt, scale=-1.0, bias=1.0)
                SC.rel("z2")
                e0 = SC.get("e0"); e1 = SC.get("e1")
                eng('diff_e0').tensor_tensor(e0, rdt, cosb, OP.mult)
                eng('diff_e1').tensor_tensor(e1, rdt, sinb, OP.mult)
                SC.rel("rdt")
                ve.copy_predicated(e0, mask, d0)
                SC.rel("d0")
                ve.copy_predicated(e1, mask, d1)
                SC.rel("d1")
                e2 = u0db
                ve.copy_predicated(e2, mask, d2p)
                SC.rel("d2p")

                er = eng('recon')
                H = SC.get("H"); Ht = SC.get("Ht"); G = SC.get("G")
                er.tensor_tensor(H, u, e0, OP.mult)
                er.tensor_tensor(Ht, by, e1, OP.mult)
                er.tensor_tensor(H, H, Ht, OP.add)
                er.tensor_tensor(G, H, rb, OP.mult)
                SC.rel("rb")
                er.tensor_tensor(G, G, e2, OP.subtract)
                gx = SC.get("gx"); gy = SC.get("gy"); gz = SC.get("gz")
                er.tensor_tensor(gx, bx, G, OP.mult)
                eng('out0').tensor_tensor(OUTv[:, :, 0], e0, gx, OP.subtract)
                SC.rel("e0")
                er.tensor_tensor(gy, sgn, e1, OP.mult)
                SC.rel("sgn")
                er.tensor_tensor(Ht, by, G, OP.mult)
                eng('out1').tensor_tensor(OUTv[:, :, 1], gy, Ht, OP.subtract)
                er.tensor_tensor(gz, e2, bz, OP.mult)
                eng('out2').tensor_tensor(OUTv[:, :, 2], gz, H, OP.subtract)
                SC.rel("bx", "by", "bz", "u", "e1", "H", "Ht", "G",
                       "gx", "gy", "gz")

                nc.sync.dma_start(wo_r[tg], OUT[:])

            rts = {}
            for t in range(T + 1):
                if t < T:
                    rts[t] = phase_a(t)
                if t >= 1:
                    phase_b(t - 1, rts.pop(t - 1), SCs[(t - 1) % nscsets])

    nc.compile()
    return nc


# ---------------- host runner (self-contained deliverable) ----------------
NCORES = 8
_CACHE = {}


def _get_module(R):
    if R not in _CACHE:
        _CACHE[R] = build_module(R, F=512, num_devices=NCORES, nscsets=2,
                                 io_bufs=2, a_bufs=2, ring=3)
    return _CACHE[R]


def kernel(wi, n, albedo, metallic, alpha_x, alpha_y, sample):
    """Full-input MultiLobeSGGX sample(): shards rays across 8 NeuronCores,
    runs the Bass kernel, gathers the full [N,3] float32 output.
    alpha_y is unused (the module asserts alpha_x == alpha_y)."""
    from concourse.bass_utils import run_bass_kernel_spmd
    wi = np.ascontiguousarray(wi, dtype=np.float32)
    n = np.ascontiguousarray(n, dtype=np.float32)
    albedo = np.ascontiguousarray(albedo, dtype=np.float32)
    metallic = np.ascontiguousarray(metallic, dtype=np.float32)
    alpha_x = np.ascontiguousarray(alpha_x, dtype=np.float32)
    sample = np.ascontiguousarray(sample, dtype=np.float32)
    Nf = wi.shape[0]
    R = Nf // NCORES
    nc = _get_module(R)
    in_maps = []
    for c in range(NCORES):
        s = slice(c * R, (c + 1) * R)
        in_maps.append({
            "wi": wi[s], "n": n[s], "albedo": albedo[s],
            "metallic": metallic[s], "alpha_x": alpha_x[s], "sample": sample[s],
        })
    res = run_bass_kernel_spmd(nc, in_maps, core_ids=list(range(NCORES)))
    return np.concatenate([res.results[c]["wo"] for c in range(NCORES)], axis=0)
